# revision 1
# baseline (speedup 1.0000x reference)
"""ChunkRanker Bass kernel for Trainium2, 8-core data-parallel.

Math per chunk n (chunks: [4096, 128, 64] f32):
  flat = chunks[n].reshape(8192)
  std  = std(flat, ddof=1)
  realism = std<0.01 ? 10*std : (std>0.5 ? 0.5/std : 1-|std-0.1|)
  ctx    = previous_context[-10:].flatten()            # [640]
  starts = flat[:640]
  boundary = dot(starts, ctx) / max(|starts|*|ctx|, 1e-8)
  score = realism + 0.15 + 0.2*boundary

Sharding: leading chunk axis split 8 ways (512 chunks/core); ctx broadcast.
Per-core layout: chunk-tiles of [128 partitions = chunks, 8192 free = chunk
elements], loaded as contiguous 4 MB HBM->SBUF DMAs (the last tile is split
into two 2 MB stages so the post-DMA latency is one half-stage, not a full
tile). Two full passes over the data are needed (sum and sum-of-squares);
they are split across the two line-rate engines so neither outruns DMA:
ACT does ACTIVATE(Square, accum_out) over all 8192 plus ACTIVATE(Copy,
accum_out) over a 1536-element slice of the plain sum; DVE reduces the
remaining 6656 elements (TENSOR_SCALAR cache-reduce, 1x) plus the two
640-element boundary terms (dot with ctx, |starts|^2). At ~8.7 us/tile per
engine vs 10.35 us/tile of DMA, the kernel stays DMA-bound.
A dummy sqrt at kernel start pins the "sqrt_and_others" ACT table set (it
contains Square and Copy too), so no table switch lands on the tail.
The scalar tail (std, piecewise realism, cosine denom) runs once on [128, 4].
"""

import numpy as np

import concourse.bacc as bacc
import concourse.bass as bass
import concourse.mybir as mybir
import concourse.tile as tile
from concourse.bass_utils import run_bass_kernel_spmd

N_CORES = 8
N_TOTAL = 4096
N_LOC = N_TOTAL // N_CORES  # 512 chunks per core
P = 128                     # chunks per tile (partition dim)
T = N_LOC // P              # 4 chunk-tiles per core
D = 128 * 64                # 8192 elements per chunk
S = 10 * 64                 # 640 boundary elements
EPS = 1e-8

# (tile_idx, slot, elem_lo, elem_hi) DMA/compute pieces. The compute engines
# run nearly saturated (~39us vs ~41us of DMA), so ramp-in and ramp-out
# latency land 1:1 on the critical path: tile 0 starts with small pieces so
# ACT's first Square begins ~9us earlier, and tile 3 ends with small pieces
# so only a 1 MB stage separates the last DMA from the tail.
Q = D // 4  # 2048 elements = 1 MB piece
PIECES = [
    (0, 0, 0, Q), (0, 1, Q, 2 * Q), (0, 2, 2 * Q, D),
    (1, 0, 0, D),
    (2, 0, 0, D),
    (3, 0, 0, D // 2), (3, 1, D // 2, 3 * Q), (3, 2, 3 * Q, D),
]
# ACT takes 3/16 of each piece's plain sum (balances ACT vs DVE totals).
ACT_SHARE = {D: 1536, D // 2: 768, Q: 384}

F32 = mybir.dt.float32
ALU = mybir.AluOpType
ACTF = mybir.ActivationFunctionType


def _build() -> bass.Bass:
    nc = bacc.Bacc(
        "TRN2", target_bir_lowering=False, debug=False, num_devices=N_CORES
    )
    x = nc.dram_tensor("chunks", [N_LOC, 128, 64], F32, kind="ExternalInput")
    ctx_in = nc.dram_tensor("ctx", [S], F32, kind="ExternalInput")
    out = nc.dram_tensor("out", [P, T], F32, kind="ExternalOutput")

    xf = x[:].rearrange("(t p) r f -> t p (r f)", p=P)  # [T, 128, 8192]

    with tile.TileContext(nc) as tc:
        with (
            tc.tile_pool(name="main", bufs=5) as main,
            tc.tile_pool(name="small", bufs=1) as small,
        ):
            # Pin the sqrt_and_others ACT table set (covers Square/Copy too)
            # before any Square runs, so the tail's sqrt needs no table load.
            warm = small.tile([P, 1], F32)
            nc.vector.memset(warm, 1.0)
            nc.scalar.activation(out=warm, in_=warm, func=ACTF.Sqrt)

            # ctx broadcast to all 128 partitions (HWDGE; gpsimd/SWDGE here
            # costs a 14 us GpSimd drain while the big loads hog the SDMAs)
            ctxb = small.tile([P, S], F32)
            cap = ctx_in[:]
            nc.sync.dma_start(
                out=ctxb,
                in_=bass.AP(tensor=cap.tensor, offset=cap.offset, ap=[[0, P], *cap.ap]),
            )

            # Per-piece accumulators laid out [128, tile, slot]; unused slots
            # stay zero so a single X-axis reduce folds slots into per-tile.
            NS = 3
            sumsq5 = small.tile([P, T * NS], F32)  # ACT: sum of squares
            suma5 = small.tile([P, T * NS], F32)   # ACT: plain-sum slice
            sumb5 = small.tile([P, T * NS], F32)   # DVE: plain-sum slice
            nc.vector.memset(sumsq5, 0.0)
            nc.vector.memset(suma5, 0.0)
            nc.vector.memset(sumb5, 0.0)
            nums = small.tile([P, T], F32)
            startsqs = small.tile([P, T], F32)

            # Accum ops must write a full-size `out` nobody reads; point them
            # at a [P,1] tile with a stride-0 AP so they cost no SBUF.
            dump_act = small.tile([P, 1], F32)
            dump_dve = small.tile([P, 1], F32)

            for t, s, lo, hi in PIECES:
                i = t * NS + s
                n = hi - lo
                za = ACT_SHARE[n]       # ACT's share of the plain sum
                xt = main.tile([P, n], F32, tag="xt")
                nc.sync.dma_start(out=xt, in_=xf[t][:, lo:hi])
                # ACT: per-chunk partial sum of squares over the whole piece
                nc.scalar.activation(
                    out=dump_act.broadcast_to([P, n]), in_=xt, func=ACTF.Square,
                    accum_out=sumsq5[:, i : i + 1],
                )
                # ACT: plain sum of the trailing za elements (Copy + accum)
                nc.scalar.activation(
                    out=dump_act.broadcast_to([P, za]), in_=xt[:, n - za :],
                    func=ACTF.Copy,
                    accum_out=suma5[:, i : i + 1],
                )
                # DVE: plain sum of the leading n-za elements
                nc.vector.tensor_scalar(
                    out=dump_dve.broadcast_to([P, n - za]), in0=xt[:, : n - za],
                    scalar1=1.0, scalar2=None,
                    op0=ALU.mult, op1=ALU.add,
                    accum_out=sumb5[:, i : i + 1],
                )
                if lo == 0:
                    # DVE: dot(starts, ctx) per chunk
                    nc.vector.scalar_tensor_tensor(
                        out=dump_dve.broadcast_to([P, S]), in0=xt[:, :S],
                        scalar=1.0, in1=ctxb,
                        op0=ALU.mult, op1=ALU.mult,
                        accum_out=nums[:, t : t + 1],
                    )
                    # DVE: |starts|^2 per chunk
                    nc.vector.scalar_tensor_tensor(
                        out=dump_dve.broadcast_to([P, S]), in0=xt[:, :S],
                        scalar=1.0, in1=xt[:, :S],
                        op0=ALU.mult, op1=ALU.mult,
                        accum_out=startsqs[:, t : t + 1],
                    )

            # ---- tail on [128, T] ----
            # |ctx|^2, identical value on every partition
            cn2 = small.tile([P, 1], F32)
            nc.vector.scalar_tensor_tensor(
                out=dump_dve.broadcast_to([P, S]), in0=ctxb, scalar=1.0, in1=ctxb,
                op0=ALU.mult, op1=ALU.mult, accum_out=cn2,
            )
            # total sums: ACT slice + DVE slice, then fold the per-piece slots
            # (unused slots are zero) with one X-axis reduce each.
            sums5 = small.tile([P, T * NS], F32)
            nc.vector.tensor_tensor(out=sums5, in0=suma5, in1=sumb5, op=ALU.add)
            sums = small.tile([P, T], F32)
            nc.vector.tensor_reduce(
                out=sums, in_=sums5[:].rearrange("p (t s) -> p t s", s=NS),
                axis=mybir.AxisListType.X, op=ALU.add,
            )
            sumsqs = small.tile([P, T], F32)
            nc.vector.tensor_reduce(
                out=sumsqs, in_=sumsq5[:].rearrange("p (t s) -> p t s", s=NS),
                axis=mybir.AxisListType.X, op=ALU.add,
            )

            # var*(D-1) = sumsq - sum^2/D ; std = sqrt(v1 / (D-1))
            t0 = small.tile([P, T], F32)
            nc.vector.scalar_tensor_tensor(
                out=t0, in0=sums, scalar=1.0 / D, in1=sums,
                op0=ALU.mult, op1=ALU.mult,
            )
            v1 = small.tile([P, T], F32)
            nc.vector.tensor_tensor(out=v1, in0=sumsqs, in1=t0, op=ALU.subtract)
            std = small.tile([P, T], F32)
            nc.scalar.activation(
                out=std, in_=v1, func=ACTF.Sqrt, scale=1.0 / (D - 1),
            )

            # piecewise realism (+0.15 regime term folded into each branch)
            b1 = small.tile([P, T], F32)
            nc.vector.tensor_scalar(
                out=b1, in0=std, scalar1=10.0, scalar2=0.15,
                op0=ALU.mult, op1=ALU.add,
            )
            rec = small.tile([P, T], F32)
            nc.vector.reciprocal(out=rec, in_=std)
            b2 = small.tile([P, T], F32)
            nc.vector.tensor_scalar(
                out=b2, in0=rec, scalar1=0.5, scalar2=0.15,
                op0=ALU.mult, op1=ALU.add,
            )
            d1 = small.tile([P, T], F32)
            nc.vector.tensor_scalar(
                out=d1, in0=std, scalar1=0.1, scalar2=None, op0=ALU.subtract,
            )
            aab = small.tile([P, T], F32)
            nc.vector.scalar_tensor_tensor(
                out=aab, in0=d1, scalar=-1.0, in1=d1, op0=ALU.mult, op1=ALU.max,
            )
            b3 = small.tile([P, T], F32)
            nc.vector.tensor_scalar(
                out=b3, in0=aab, scalar1=-1.0, scalar2=1.15,
                op0=ALU.mult, op1=ALU.add,
            )
            m1 = small.tile([P, T], mybir.dt.uint8)
            nc.vector.tensor_scalar(
                out=m1, in0=std, scalar1=0.01, scalar2=None, op0=ALU.is_lt,
            )
            m2 = small.tile([P, T], mybir.dt.uint8)
            nc.vector.tensor_scalar(
                out=m2, in0=std, scalar1=0.5, scalar2=None, op0=ALU.is_gt,
            )
            r1 = small.tile([P, T], F32)
            nc.vector.select(out=r1, mask=m2, on_true=b2, on_false=b3)
            realism = small.tile([P, T], F32)
            nc.vector.select(out=realism, mask=m1, on_true=b1, on_false=r1)

            # boundary = num / max(sqrt(startsq * |ctx|^2), eps)
            d2 = small.tile([P, T], F32)
            nc.vector.tensor_scalar(
                out=d2, in0=startsqs, scalar1=cn2, scalar2=None, op0=ALU.mult,
            )
            den = small.tile([P, T], F32)
            nc.scalar.activation(out=den, in_=d2, func=ACTF.Sqrt)
            den2 = small.tile([P, T], F32)
            nc.vector.tensor_scalar(
                out=den2, in0=den, scalar1=EPS, scalar2=None, op0=ALU.max,
            )
            rden = small.tile([P, T], F32)
            nc.vector.reciprocal(out=rden, in_=den2)
            bnd = small.tile([P, T], F32)
            nc.vector.tensor_tensor(out=bnd, in0=nums, in1=rden, op=ALU.mult)

            final = small.tile([P, T], F32)
            nc.vector.scalar_tensor_tensor(
                out=final, in0=bnd, scalar=0.2, in1=realism,
                op0=ALU.mult, op1=ALU.add,
            )
            nc.sync.dma_start(out=out[:], in_=final)
    nc.compile()
    return nc


_NC_CACHE = None


def _get_nc() -> bass.Bass:
    global _NC_CACHE
    if _NC_CACHE is None:
        _NC_CACHE = _build()
    return _NC_CACHE


def run(inputs: dict, trace: bool = False, **kw):
    """Returns (output [4096] f32, BassKernelResults)."""
    chunks = np.ascontiguousarray(np.asarray(inputs["chunks"], dtype=np.float32))
    pc = np.asarray(inputs["previous_context"], dtype=np.float32)
    ctx = np.ascontiguousarray(pc[-10:].reshape(-1))
    assert chunks.shape == (N_TOTAL, 128, 64)
    assert ctx.shape == (S,)

    nc = _get_nc()
    in_maps = [
        {"chunks": chunks[c * N_LOC : (c + 1) * N_LOC], "ctx": ctx}
        for c in range(N_CORES)
    ]
    res = run_bass_kernel_spmd(nc, in_maps, core_ids=list(range(N_CORES)),
                               trace=trace, **kw)
    # out[p, t] = score of local chunk t*128+p -> transpose to chunk order
    full = np.concatenate([r["out"].T.reshape(-1) for r in res.results])
    return full.astype(np.float32), res


def kernel(**inputs) -> np.ndarray:
    return run(inputs)[0]



# revision 4
# speedup vs baseline: 1.0592x; 1.0592x over previous
"""ChunkRanker Bass kernel for Trainium2, 8-core data-parallel.

Math per chunk n (chunks: [4096, 128, 64] f32):
  flat = chunks[n].reshape(8192)
  std  = std(flat, ddof=1)
  realism = std<0.01 ? 10*std : (std>0.5 ? 0.5/std : 1-|std-0.1|)
  ctx    = previous_context[-10:].flatten()            # [640]
  starts = flat[:640]
  boundary = dot(starts, ctx) / max(|starts|*|ctx|, 1e-8)
  score = realism + 0.15 + 0.2*boundary

Sharding: leading chunk axis split 8 ways (512 chunks/core); ctx broadcast.

Per-core design (v2): the whole 16 MiB local input fits in SBUF
(128 KiB/partition of ~208 usable), so every DMA piece gets its own
buffer and ALL input DMAs are issued up-front -- the DMA stream runs at
pure HBM rate with no buffer-reuse coupling to compute.  Trace analysis
of the v1 kernel showed DMA busy ~41us but spread over ~60-68us because
the 5-buffer pool gated DMA on compute and vice versa.

Engine split (measured rates: ACT 0.833 ns/col marginal, DVE 1.042):
  ACT: Square+accum over all 8192 cols x 4 tiles (sumsq) plus Copy+accum
       over cols [0:2560] of tiles 1,2 (part of the plain sum).  ~36 us.
  DVE: plain-sum of the rest + the two 640-col boundary dots per tile
       + the scalar tail.  ~37 us.
Both sit under the ~47-52 us HBM-bound DMA window, so compute hides
behind DMA and the kernel ends ~2-4 us after the last (small) piece.

Tail: one ACT visit only -- DVE packs var*(scale) and |starts|^2*|ctx|^2
into one [128, 2T] tile, ACT does a single Sqrt, DVE does max/reciprocal
once over the pair, then the piecewise-realism chain.  cn2 (|ctx|^2) is
computed early, off the critical path.
"""

import numpy as np

import concourse.bacc as bacc
import concourse.bass as bass
import concourse.mybir as mybir
import concourse.tile as tile
from concourse.bass_utils import run_bass_kernel_spmd

N_CORES = 8
N_TOTAL = 4096
N_LOC = N_TOTAL // N_CORES  # 512 chunks per core
P = 128                     # chunks per tile (partition dim)
T = N_LOC // P              # 4 chunk-tiles per core
D = 128 * 64                # 8192 elements per chunk
S = 10 * 64                 # 640 boundary elements
EPS = 1e-8

# (tile, elem_lo, elem_hi, slot): first/last tiles split so ACT ramps in
# early and the final piece is small; middle tiles are single 4MB DMAs.
NS = 3
PIECES = [
    (0, 0, 2048, 0),
    (0, 2048, 4096, 1),
    (0, 4096, 8192, 2),
    (1, 0, 8192, 0),
    (2, 0, 8192, 0),
    (3, 0, 4096, 0),
    (3, 4096, 6144, 1),
    (3, 6144, 8192, 2),
]
ACOPY = 2560  # ACT's plain-sum share on tiles 1 and 2

F32 = mybir.dt.float32
ALU = mybir.AluOpType
ACTF = mybir.ActivationFunctionType


def _build() -> bass.Bass:
    nc = bacc.Bacc(
        "TRN2", target_bir_lowering=False, debug=False, num_devices=N_CORES
    )
    x = nc.dram_tensor("chunks", [N_LOC, 128, 64], F32, kind="ExternalInput")
    ctx_in = nc.dram_tensor("ctx", [S], F32, kind="ExternalInput")
    out = nc.dram_tensor("out", [P, T], F32, kind="ExternalOutput")

    xf = x[:].rearrange("(t p) r f -> t p (r f)", p=P)  # [T, 128, 8192]

    with tile.TileContext(nc) as tc:
        with (
            tc.tile_pool(name="data", bufs=1) as data,
            tc.tile_pool(name="small", bufs=1) as small,
        ):
            # Pin the sqrt_and_others ACT table set (covers Square/Copy too)
            # before any Square runs, so no mid-kernel table load.
            warm = small.tile([P, 1], F32)
            nc.vector.memset(warm, 1.0)
            nc.scalar.activation(out=warm, in_=warm, func=ACTF.Sqrt)

            # --- all input DMAs issued up-front; each piece owns its buffer
            xts = {}
            cap = ctx_in[:]
            ctxb = small.tile([P, S], F32)
            for i, (t, lo, hi, s) in enumerate(PIECES):
                xt = data.tile([P, hi - lo], F32, name=f"xt{i}", uniquify=False)
                xts[i] = xt
                nc.sync.dma_start(out=xt, in_=xf[t][:, lo:hi])
                if i == 0:
                    # ctx broadcast to 128 partitions; issued second so the
                    # first data piece's descriptors go out first.
                    nc.sync.dma_start(
                        out=ctxb,
                        in_=bass.AP(
                            tensor=cap.tensor, offset=cap.offset,
                            ap=[[0, P], *cap.ap],
                        ),
                    )

            # Per-piece accumulators [128, tile*slot]; unused slots stay zero
            # so one X-axis reduce folds slots into per-tile totals.
            sumsq5 = small.tile([P, T * NS], F32)  # ACT: sum of squares
            suma5 = small.tile([P, T * NS], F32)   # ACT: plain-sum share
            sumb5 = small.tile([P, T * NS], F32)   # DVE: plain-sum share
            nc.vector.memset(sumsq5, 0.0)
            nc.vector.memset(suma5, 0.0)
            nc.vector.memset(sumb5, 0.0)
            nums = small.tile([P, T], F32)
            startsqs = small.tile([P, T], F32)
            cn2 = small.tile([P, 1], F32)

            # Accum ops need a full-size `out` nobody reads; stride-0 view of
            # a [P,1] tile costs no SBUF.
            dump_act = small.tile([P, 1], F32)
            dump_dve = small.tile([P, 1], F32)
            # Probe: real (non-broadcast) out for one DVE sum, to see if the
            # 2x_2p DVE perf mode engages when the store AP is packed.
            scr = small.tile([P, D - ACOPY], F32)

            # --- ACT: sum of squares everywhere + Copy-sum share on t1/t2
            for i, (t, lo, hi, s) in enumerate(PIECES):
                xt = xts[i]
                n = hi - lo
                nc.scalar.activation(
                    out=dump_act.broadcast_to([P, n]), in_=xt, func=ACTF.Square,
                    accum_out=sumsq5[:, t * NS + s : t * NS + s + 1],
                )
                if t in (1, 2):
                    nc.scalar.activation(
                        out=dump_act.broadcast_to([P, ACOPY]),
                        in_=xt[:, :ACOPY], func=ACTF.Copy,
                        accum_out=suma5[:, t * NS : t * NS + 1],
                    )

            # --- DVE: plain sums + boundary dots
            def dve_sum(xt, lo, hi, slot, probe=None):
                n = hi - lo
                if probe == "reduce":
                    nc.vector.tensor_reduce(
                        out=sumb5[:, slot : slot + 1], in_=xt[:, lo:hi],
                        axis=mybir.AxisListType.X, op=ALU.add,
                    )
                else:
                    o = (scr[:, :n] if probe == "realout"
                         else dump_dve.broadcast_to([P, n]))
                    nc.vector.tensor_scalar(
                        out=o, in0=xt[:, lo:hi],
                        scalar1=1.0, scalar2=None, op0=ALU.mult, op1=ALU.add,
                        accum_out=sumb5[:, slot : slot + 1],
                    )

            def dve_dots(xt, t):
                nc.vector.scalar_tensor_tensor(
                    out=dump_dve.broadcast_to([P, S]), in0=xt[:, :S],
                    scalar=1.0, in1=ctxb, op0=ALU.mult, op1=ALU.mult,
                    accum_out=nums[:, t : t + 1],
                )
                nc.vector.scalar_tensor_tensor(
                    out=dump_dve.broadcast_to([P, S]), in0=xt[:, :S],
                    scalar=1.0, in1=xt[:, :S], op0=ALU.mult, op1=ALU.mult,
                    accum_out=startsqs[:, t : t + 1],
                )

            dve_sum(xts[0], 0, 2048, 0)
            dve_dots(xts[0], 0)
            # |ctx|^2 early, off the tail's critical path
            nc.vector.scalar_tensor_tensor(
                out=dump_dve.broadcast_to([P, S]), in0=ctxb, scalar=1.0,
                in1=ctxb, op0=ALU.mult, op1=ALU.mult, accum_out=cn2,
            )
            dve_sum(xts[1], 0, 2048, 1)       # piece-local cols
            dve_sum(xts[2], 0, 4096, 2)
            dve_sum(xts[3], ACOPY, D, 1 * NS + 1, probe="realout")
            dve_dots(xts[3], 1)
            dve_sum(xts[4], ACOPY, D, 2 * NS + 1, probe="reduce")
            dve_dots(xts[4], 2)
            dve_sum(xts[5], 0, 4096, 3 * NS + 0)
            dve_dots(xts[5], 3)
            dve_sum(xts[6], 0, 2048, 3 * NS + 1)
            dve_sum(xts[7], 0, 2048, 3 * NS + 2)

            # ---- tail on [128, T] ----
            sums5 = small.tile([P, T * NS], F32)
            nc.vector.tensor_tensor(out=sums5, in0=suma5, in1=sumb5, op=ALU.add)
            sums = small.tile([P, T], F32)
            nc.vector.tensor_reduce(
                out=sums, in_=sums5[:].rearrange("p (t s) -> p t s", s=NS),
                axis=mybir.AxisListType.X, op=ALU.add,
            )
            sumsqs = small.tile([P, T], F32)
            nc.vector.tensor_reduce(
                out=sumsqs, in_=sumsq5[:].rearrange("p (t s) -> p t s", s=NS),
                axis=mybir.AxisListType.X, op=ALU.add,
            )

            # cat = [ var_ddof1 | startsq*|ctx|^2 ], one ACT Sqrt for both
            cat = small.tile([P, 2 * T], F32)
            t0 = small.tile([P, T], F32)
            nc.vector.scalar_tensor_tensor(
                out=t0, in0=sums, scalar=1.0 / (float(D) * (D - 1)), in1=sums,
                op0=ALU.mult, op1=ALU.mult,
            )
            nc.vector.scalar_tensor_tensor(
                out=cat[:, 0:T], in0=sumsqs, scalar=1.0 / (D - 1), in1=t0,
                op0=ALU.mult, op1=ALU.subtract,
            )
            nc.vector.tensor_scalar(
                out=cat[:, T : 2 * T], in0=startsqs, scalar1=cn2, scalar2=None,
                op0=ALU.mult,
            )
            sqcat = small.tile([P, 2 * T], F32)
            nc.scalar.activation(out=sqcat, in_=cat, func=ACTF.Sqrt)

            # clamp + reciprocal once for both std and den
            den2 = small.tile([P, 2 * T], F32)
            nc.vector.tensor_scalar(
                out=den2, in0=sqcat, scalar1=EPS, scalar2=None, op0=ALU.max,
            )
            rboth = small.tile([P, 2 * T], F32)
            nc.vector.reciprocal(out=rboth, in_=den2)
            std = sqcat[:, 0:T]

            # piecewise realism (+0.15 regime term folded into each branch)
            b1 = small.tile([P, T], F32)
            nc.vector.tensor_scalar(
                out=b1, in0=std, scalar1=10.0, scalar2=0.15,
                op0=ALU.mult, op1=ALU.add,
            )
            b2 = small.tile([P, T], F32)
            nc.vector.tensor_scalar(
                out=b2, in0=rboth[:, 0:T], scalar1=0.5, scalar2=0.15,
                op0=ALU.mult, op1=ALU.add,
            )
            d1 = small.tile([P, T], F32)
            nc.vector.tensor_scalar(
                out=d1, in0=std, scalar1=0.1, scalar2=None, op0=ALU.subtract,
            )
            aab = small.tile([P, T], F32)
            nc.vector.scalar_tensor_tensor(
                out=aab, in0=d1, scalar=-1.0, in1=d1, op0=ALU.mult, op1=ALU.max,
            )
            b3 = small.tile([P, T], F32)
            nc.vector.tensor_scalar(
                out=b3, in0=aab, scalar1=-1.0, scalar2=1.15,
                op0=ALU.mult, op1=ALU.add,
            )
            m1 = small.tile([P, T], mybir.dt.uint8)
            nc.vector.tensor_scalar(
                out=m1, in0=std, scalar1=0.01, scalar2=None, op0=ALU.is_lt,
            )
            m2 = small.tile([P, T], mybir.dt.uint8)
            nc.vector.tensor_scalar(
                out=m2, in0=std, scalar1=0.5, scalar2=None, op0=ALU.is_gt,
            )
            r1 = small.tile([P, T], F32)
            nc.vector.select(out=r1, mask=m2, on_true=b2, on_false=b3)
            realism = small.tile([P, T], F32)
            nc.vector.select(out=realism, mask=m1, on_true=b1, on_false=r1)

            bnd = small.tile([P, T], F32)
            nc.vector.tensor_tensor(
                out=bnd, in0=nums, in1=rboth[:, T : 2 * T], op=ALU.mult,
            )
            final = small.tile([P, T], F32)
            nc.vector.scalar_tensor_tensor(
                out=final, in0=bnd, scalar=0.2, in1=realism,
                op0=ALU.mult, op1=ALU.add,
            )
            nc.sync.dma_start(out=out[:], in_=final)
    nc.compile()
    return nc


_NC_CACHE = None


def _get_nc() -> bass.Bass:
    global _NC_CACHE
    if _NC_CACHE is None:
        _NC_CACHE = _build()
    return _NC_CACHE


def run(inputs: dict, trace: bool = False, **kw):
    """Returns (output [4096] f32, BassKernelResults)."""
    chunks = np.ascontiguousarray(np.asarray(inputs["chunks"], dtype=np.float32))
    pc = np.asarray(inputs["previous_context"], dtype=np.float32)
    ctx = np.ascontiguousarray(pc[-10:].reshape(-1))
    assert chunks.shape == (N_TOTAL, 128, 64)
    assert ctx.shape == (S,)

    nc = _get_nc()
    in_maps = [
        {"chunks": chunks[c * N_LOC : (c + 1) * N_LOC], "ctx": ctx}
        for c in range(N_CORES)
    ]
    res = run_bass_kernel_spmd(nc, in_maps, core_ids=list(range(N_CORES)),
                               trace=trace, **kw)
    # out[p, t] = score of local chunk t*128+p -> transpose to chunk order
    full = np.concatenate([r["out"].T.reshape(-1) for r in res.results])
    return full.astype(np.float32), res


def kernel(**inputs) -> np.ndarray:
    return run(inputs)[0]
